# revision 2
# baseline (speedup 1.0000x reference)
"""Lowpass (leaky integrator) scan kernel for Trainium2, 8 NeuronCores.

Recurrence (per feature n, per batch b):
    a_n = exp(-dt / max(tau_n, 1e-8))
    x_t = a_n * x_{t-1} + (1 - a_n) * u_t,   x_{-1} = initial_level_n

v8: fp16 I/O, 4x-downsampled scan, GpSimd left idle (it shares SBUF
ports with the DVE - concurrent Pool ops were measured to double DVE
op latency), phase p2 on the TensorEngine.
  - 8 tiles/core of [128, 2048] fp16 (512 KiB DMAs, prefetch-bounded).
    Loads, consts and stores all ride the Sync HWDGE ring; the ScalarE
    stream is the (1-a)u pre-scale (sections 0-1 only), the 1-col chain
    copies, and the p2 PSUM->SBUF copy-out.
  - TensorE: the 4-step forcing w = sum_j diag(a^(3-j)(1-a)) @ u_j in
    PSUM, plus p2 = diag(a) @ p1 + diag(1-a) @ u2_raw in a second PSUM
    bank (lagged one tile behind the DVE chain).
  - DVE stream per tile: scan (reads w from PSUM), p0, p1 - self-paced
    fused ops, no incoming cross-engine hops, no SBUF-port contention.
  - xout layout: col 31 = chain-in, [base|p0|p1|p2] from col 32 so the
    store source is 64B aligned.
"""

import numpy as np
from contextlib import ExitStack

import concourse.bacc as bacc
import concourse.mybir as mybir
import concourse.tile as tile
from concourse.bass_utils import run_bass_kernel_spmd

DT = 0.001
B, T, N = 32, 4096, 128
NCORES = 8
BC = B // NCORES      # batches per core
Q = 4                 # time downsample factor of the scan
TB = 2048             # time columns per tile
TBq = TB // Q         # scan columns (groups) per tile
NBLK = T // TB        # tiles per batch
NT = BC * NBLK        # tiles per core
PF = 3                # load prefetch depth (tiles)
PAD = 32              # xout column pad for store alignment
ND = 5                # diag tables: a^3(1-a), a^2(1-a), a(1-a), (1-a), a

_F32 = mybir.dt.float32
_F16 = mybir.dt.float16
_MUL = mybir.AluOpType.mult
_ADD = mybir.AluOpType.add


def build_nc():
    nc = bacc.Bacc("TRN2", target_bir_lowering=False, debug=False)
    u = nc.declare_dram_parameter("u", [NT, N, TB], _F16, isOutput=False)
    cols_in = nc.declare_dram_parameter("cols3", [3, N], _F32, isOutput=False)
    a4b_in = nc.declare_dram_parameter("a4b", [N, TBq], _F32, isOutput=False)
    diag_in = nc.declare_dram_parameter("diag", [N, ND * N], _F16, isOutput=False)
    y = nc.declare_dram_parameter("y", [NT, N, TB], _F16, isOutput=True)

    with tile.TileContext(nc) as tc, ExitStack() as ctx:
        const = ctx.enter_context(tc.tile_pool(name="const", bufs=1))
        uin = ctx.enter_context(tc.tile_pool(name="uin", bufs=6))
        upr = ctx.enter_context(tc.tile_pool(name="upr", bufs=4))
        xo = ctx.enter_context(tc.tile_pool(name="xo", bufs=6))
        pp = ctx.enter_context(tc.psum_pool(name="pp", bufs=4))
        p2p = ctx.enter_context(tc.psum_pool(name="p2p", bufs=3))

        # First input tile ahead of everything on the Sync ring.
        uts = {0: uin.tile([128, TB], _F16, name="ut")}
        nc.sync.dma_start(uts[0][:], u[0])

        cols = const.tile([128, 3], _F32)   # [:,0]=a, [:,1]=oma, [:,2]=x0
        a4b = const.tile([128, TBq], _F32)
        diag = const.tile([128, ND * N], _F16)
        nc.sync.dma_start(cols[:], cols_in[:].rearrange("o n -> n o"))
        nc.sync.dma_start(a4b[:], a4b_in[:])
        nc.sync.dma_start(diag[:], diag_in[:])
        acol = cols[:, 0:1]
        omacol = cols[:, 1:2]
        x0col = cols[:, 2:3]
        for ti in range(1, min(PF, NT)):
            uts[ti] = uin.tile([128, TB], _F16, name="ut")
            nc.sync.dma_start(uts[ti][:], u[ti])

        prev = [None] * BC   # last xout tile per batch (for chaining)
        pend = []            # (ti, xout, ut, up) pending p2 + store
        for ti in range(NT):
            kb, b = divmod(ti, BC)
            ut = uts.pop(ti)
            if ti + PF < NT:
                uts[ti + PF] = uin.tile([128, TB], _F16, name="ut")
                nc.sync.dma_start(uts[ti + PF][:], u[ti + PF])

            # ScalarE: u' = (1-a)*u for phase sections 0..1 only
            up = upr.tile([128, 2 * TBq], _F16, name="up")
            nc.scalar.mul(up[:], ut[:, 0:2 * TBq], omacol)

            # TensorE: w = sum_j diag(a^(3-j)*(1-a)) @ u_j  -> PSUM fp32
            wps = pp.tile([128, TBq], _F32, name="wps")
            for j in range(Q):
                nc.tensor.matmul(
                    wps[:], diag[:, j * N:(j + 1) * N],
                    ut[:, j * TBq:(j + 1) * TBq],
                    start=(j == 0), stop=(j == Q - 1),
                )

            # xout: col PAD-1 = chain-in, then [base|p0|p1|p2] from col PAD
            xout = xo.tile([128, PAD + TB], _F16, name="xout")
            if kb == 0:
                nc.scalar.copy(xout[:, PAD - 1:PAD], x0col)
            else:
                nc.scalar.copy(
                    xout[:, PAD - 1:PAD], prev[b][:, PAD + TBq - 1:PAD + TBq]
                )

            # DVE stream per tile: scan -> p0 -> p1
            nc.vector.tensor_tensor_scan(
                xout[:, PAD:PAD + TBq], a4b[:], wps[:],
                xout[:, PAD - 1:PAD], _MUL, _ADD,
            )
            nc.vector.scalar_tensor_tensor(
                xout[:, PAD + TBq:PAD + 2 * TBq], xout[:, PAD - 1:PAD + TBq - 1],
                acol, up[:, 0:TBq], _MUL, _ADD,
            )
            nc.vector.scalar_tensor_tensor(
                xout[:, PAD + 2 * TBq:PAD + 3 * TBq],
                xout[:, PAD + TBq:PAD + 2 * TBq],
                acol, up[:, TBq:2 * TBq], _MUL, _ADD,
            )
            prev[b] = xout
            pend.append((ti, xout, ut))

            # lagged one tile: p2 = diag(a)@p1 + diag(1-a)@u2_raw on PE,
            # ACT copies it out of PSUM, then the whole tile stores (Sync)
            if len(pend) > 1:
                _flush(nc, tc, y, diag, p2p, pend.pop(0))
        while pend:
            _flush(nc, tc, y, diag, p2p, pend.pop(0))
    nc.compile()
    return nc


def _flush(nc, tc, y, diag, p2p, item):
    ti, xout, ut = item
    p2ps = p2p.tile([128, TBq], _F32, name="p2ps")
    nc.tensor.matmul(
        p2ps[:], diag[:, 4 * N:5 * N],
        xout[:, PAD + 2 * TBq:PAD + 3 * TBq], start=True, stop=False,
    )
    nc.tensor.matmul(
        p2ps[:], diag[:, 3 * N:4 * N],
        ut[:, 2 * TBq:3 * TBq], start=False, stop=True,
    )
    nc.scalar.copy(xout[:, PAD + 3 * TBq:PAD + 4 * TBq], p2ps[:])
    nc.sync.dma_start(y[ti], xout[:, PAD:PAD + TB])


_NC = None


def _get_nc():
    global _NC
    if _NC is None:
        _NC = build_nc()
    return _NC


def _coeffs(initial_level, tau):
    tau = np.asarray(tau, dtype=np.float32)
    x0 = np.asarray(initial_level, dtype=np.float32)
    # fp32 exp via jax-on-CPU so `a` is bit-identical to the reference's.
    try:
        import jax

        with jax.default_device(jax.local_devices(backend="cpu")[0]):
            a = np.asarray(
                jax.numpy.exp(-DT / jax.numpy.maximum(tau, 1e-8)),
                dtype=np.float32,
            )
    except Exception:
        a = np.exp(-np.float32(DT) / np.maximum(tau, np.float32(1e-8))).astype(
            np.float32
        )
    oma = (np.float32(1.0) - a).astype(np.float32)
    a4 = (a * a * a * a).astype(np.float32)
    cols3 = np.concatenate([a, oma, x0], axis=0).astype(np.float32)  # [3, N]
    diag = np.zeros((N, ND * N), np.float16)
    idx = np.arange(N)
    for j in range(Q):
        diag[idx, j * N + idx] = (a[0] ** (Q - 1 - j) * oma[0]).astype(np.float16)
    diag[idx, 4 * N + idx] = a[0].astype(np.float16)
    a4b = np.ascontiguousarray(np.broadcast_to(a4.reshape(N, 1), (N, TBq)))
    return cols3, a4b, diag


def make_in_maps(inputs, initial_level, tau):
    cols3, a4b, diag = _coeffs(initial_level, tau)
    u = np.asarray(inputs, dtype=np.float32)
    # slab[ti=kb*BC+b][n, j*TBq + k] = u[b, kb*TB + 4k+j, n]
    v = u.reshape(B, NBLK, TBq, Q, N).transpose(0, 1, 4, 3, 2)  # [b,kb,n,j,k]
    v = np.ascontiguousarray(v.astype(np.float16)).reshape(B, NBLK, N, TB)
    maps = []
    for i in range(NCORES):
        vc = v[i * BC:(i + 1) * BC]                      # [BC, NBLK, N, TB]
        uc = np.ascontiguousarray(
            vc.transpose(1, 0, 2, 3).reshape(NT, N, TB)  # ti = kb*BC + b
        )
        maps.append({"u": uc, "cols3": cols3, "a4b": a4b, "diag": diag})
    return maps


def unshard_out(res):
    # y slab sections are [base|p0|p1|p2] = phases j=[3,0,1,2]
    out = np.stack([res[i]["y"] for i in range(NCORES)])  # [C, NT, N, TB]
    out = out.reshape(NCORES, NBLK, BC, N, Q, TBq)
    out = out.transpose(0, 2, 1, 5, 4, 3)                 # [C, b, kb, k, sec, n]
    out = out.astype(np.float32)
    y = np.empty((NCORES, BC, NBLK, TBq, Q, N), np.float32)
    y[:, :, :, :, 3, :] = out[:, :, :, :, 0, :]
    y[:, :, :, :, 0, :] = out[:, :, :, :, 1, :]
    y[:, :, :, :, 1, :] = out[:, :, :, :, 2, :]
    y[:, :, :, :, 2, :] = out[:, :, :, :, 3, :]
    return np.ascontiguousarray(y.reshape(B, T, N))


def kernel(inputs, initial_level, tau):
    nc = _get_nc()
    in_maps = make_in_maps(inputs, initial_level, tau)
    res = run_bass_kernel_spmd(nc, in_maps, list(range(NCORES))).results
    return unshard_out(res)
